# revision 53
# baseline (speedup 1.0000x reference)
"""Causal single-head attention on 8 TRN2 NeuronCores.

Problem: x[4, 4096, 1024], W_q/W_k/W_v [1024, 64] ->
         softmax(causal(q k^T)/8) v   -> [4, 4096, 64]

Sharding: core c = (batch b = c//2, half h = c%2). Each core handles 2048
queries of its batch: h=0 the even 128-row query tiles, h=1 the odd ones
(balanced causal work). The host permutes x's rows per core (own-half tiles
first), so all 8 cores run ONE identical program; the causal structure
differences between halves are encoded in small per-core mask tensors.

Per-core pipeline (all matmuls bf16 hi/lo split, >=16-bit effective):
  A) K^T,Q^T (3-term split, packed [K|Q]) and V^T projections. Quarters 2-3
     don't need Q, so their V rides the [K|V]-packed hi term for free.
  B) max pass: S1 = Qh.Kh per query tile (+ causal masks), row-max m.
  C) value pass: S^T = Qh.Kh + Qh.Kl + Ql.Kh - m computed directly in
     [key, query] layout (the -m rides an appended ones-row of K / -m row
     of Q), exp on ScalarE -> P^T, then out^T = [V|1]^T P^T accumulated on
     PE; the appended ones-column of V yields the softmax denominator Z.
  Final: transpose out^T, multiply by 1/Z, store.

PSUM (8 banks): proj[128,512]x2 (A projections; B's late s1 reuses it) |
s[128,512]x4 (B s1 / C s3 / vps / tf) | pv[65,512]x2 (pv banks + V-
transpose staging). Emission: A quarters run against stage B's DVE-bound
row-max reduces; A3's xl-dependent matmuls are deferred past the DMA
tail; stage C runs as one chunk-level stream (C0 banks then C1 banks)
whose pv banks close early and cascade into the output tiles.
"""
import sys

sys.path.insert(0, "/opt/trn_rl_repo")

import numpy as np
import ml_dtypes

import concourse.bass as bass
import concourse.mybir as mybir
import concourse.tile as tile
from concourse import bacc
from concourse.masks import make_identity
from concourse.bass_utils import run_bass_kernel_spmd

BF = mybir.dt.bfloat16
F32 = mybir.dt.float32
BF_NP = ml_dtypes.bfloat16

P = 128
S = 4096
D = 1024
H = 64
QT = 16            # query tiles per core
KT = 32            # key tiles
QC = QT * P        # queries per core
NDC = D // P       # d-chunks
SCALE = 0.125      # 1/sqrt(64)
NEG = -1e30
N_CORES = 8
B = 4
CFG = {"order": "p2", "c_la": 4, "xin_bufs": 16, "cpb": 4}


def _build(reps: int = 1):
    nc = bacc.Bacc(None, target_bir_lowering=False)
    xh_d = nc.dram_tensor("xh", [D, S], BF, kind="ExternalInput")
    xl_d = nc.dram_tensor("xl", [D, S], BF, kind="ExternalInput")
    wa1_d = nc.dram_tensor("wa1", [P, NDC * P], BF, kind="ExternalInput")  # [Wk_h | Wq_h]
    wa3_d = nc.dram_tensor("wa3", [P, NDC * P], BF, kind="ExternalInput")  # [Wk_l | Wq_l]
    wkv_d = nc.dram_tensor("wkv", [P, NDC * P], BF, kind="ExternalInput")  # [Wk_h | Wv_h]
    wv_d = nc.dram_tensor("wv", [P, NDC * H], BF, kind="ExternalInput")    # Wv_h
    mApd_d = nc.dram_tensor("mApd", [P, 640], F32, kind="ExternalInput")  # [512 zeros | triA]
    mBpd_d = nc.dram_tensor("mBpd", [P, 640], F32, kind="ExternalInput")  # [512 zeros | mB]
    mbig_d = nc.dram_tensor("mbig", [P, 1], F32, kind="ExternalInput")  # 0 (h=0) | 3e38
    out_d = nc.dram_tensor("out", [QC, H], F32, kind="ExternalOutput")

    with tile.TileContext(nc) as tc:
        for _ in range(reps):
            _body(nc, tc, xh_d, xl_d, wa1_d, wa3_d, wkv_d, wv_d,
                  mApd_d, mBpd_d, mbig_d, out_d)
    nc.finalize()
    return nc


def _body(nc, tc, xh_d, xl_d, wa1_d, wa3_d, wkv_d, wv_d,
          mApd_d, mBpd_d, mbig_d, out_d):
    SQ = 1024          # s-quarter width

    with tc.tile_pool(name="pers", bufs=1) as pers:
        # ---------------- persistent SBUF ----------------
        KhKl = pers.tile([P, S], BF)      # rows 0:64 = Kh, 64:128 = Kl
        K65 = pers.tile([65, S], BF)      # Kh + ones row (for -m contraction)
        QhQh = pers.tile([P, QC], BF)     # Qh duplicated on both halves
        QlM = pers.tile([65, QC], BF)     # Ql + (-m) row
        V65 = pers.tile([P, KT, 65], BF)  # V tiles + ones column
        VTsb = pers.tile([H, S], BF)      # V^T staging
        M16a = pers.tile([P, QT], F32)    # col p = range-1 max
        M16b = pers.tile([P, QT], F32)    # col p = range-2 max
        M16 = pers.tile([P, QT], F32)     # col p = -max
        m16t0 = pers.tile([8, P], F32)
        m16t1 = pers.tile([8, P], F32)
        pvsb = pers.tile([65, QC], F32)   # out^T staging
        idf = pers.tile([P, P], F32)
        idb = pers.tile([P, P], BF)
        wa1 = pers.tile([P, NDC, P], BF)
        wa3 = pers.tile([P, NDC, P], BF)
        wkv = pers.tile([P, NDC, P], BF)
        wk0 = pers.tile([P, NDC, P], BF)
        wv = pers.tile([P, NDC, H], BF)
        mApd = pers.tile([P, 640], F32)
        mBpd = pers.tile([P, 640], F32)
        mbig = pers.tile([P, 1], F32)

        nc.scalar.dma_start(wa1[:], wa1_d.rearrange("p (c m) -> p c m", c=NDC))
        nc.scalar.dma_start(wa3[:], wa3_d.rearrange("p (c m) -> p c m", c=NDC))
        nc.scalar.dma_start(wkv[:], wkv_d.rearrange("p (c m) -> p c m", c=NDC))
        nc.scalar.dma_start(wv[:], wv_d.rearrange("p (c m) -> p c m", c=NDC))
        make_identity(nc, idb[:])
        # wk0 = [Wk_h | 0] built on-chip from wa1's left halves
        nc.gpsimd.tensor_copy(wk0[:, :, 0:64], wa1[:, :, 0:64])
        nc.gpsimd.memset(wk0[:, :, 64:128], 0.0)

        # PSUM: proj[128,512]x2 (A) | s[128,512]x4 (B/C/transients) | pv x1.
        with tc.tile_pool(name="pp", bufs=2, space="PSUM") as pp, \
             tc.tile_pool(name="xin", bufs=CFG["xin_bufs"]) as xin, \
             tc.tile_pool(name="ptp", bufs=3) as ptp:

            def stage_a_quarter(sq, cc_filter=None):
                c0 = sq * SQ
                qown = sq < 2      # quarter carries this core's own queries
                xts = stage_a_dma(sq)
                for cc in (0, 512):
                    stage_a_chunk(sq, c0, qown, xts, cc)

            def stage_a_dma(sq):
                # one queue, all xh of the quarter before all xl: the hi-term
                # matmuls (first in each chunk) never wait on xl
                c0 = sq * SQ
                xts = []
                for d in range(NDC):
                    xht = xin.tile([P, SQ], BF, tag="xht", bufs=CFG["xin_bufs"])
                    xlt = xin.tile([P, SQ], BF, tag="xlt", bufs=CFG["xin_bufs"])
                    nc.sync.dma_start(xht[:], xh_d[d * P:(d + 1) * P, c0:c0 + SQ])
                    xts.append((xht, xlt))
                for d in range(NDC):
                    nc.sync.dma_start(xts[d][1][:], xl_d[d * P:(d + 1) * P, c0:c0 + SQ])
                return xts

            _g1c = {}

            def stage_a_chunk(sq, c0, qown, xts, cc, part="all"):
                sl = slice(cc, cc + 512)
                if part in ("all", "hi"):
                    g1 = pp.tile([P, 512], F32, tag="proj", name=f"g1_{sq}_{cc}", bufs=2)
                    vps = (pp.tile([H, 512], F32, tag="s", name=f"vps_{sq}_{cc}",
                                   bufs=4) if qown else None)
                    _g1c[(sq, cc)] = (g1, vps)
                else:
                    g1, vps = _g1c.pop((sq, cc))
                # xh terms for every d-chunk first, then the xl terms.
                # kv-packed quarters: group opens with the 128-wide wkv term
                # and closes with the 128-wide [Wk_h|0] xl term, so start and
                # stop both span the full tile (V half accumulates +0 from
                # the zero columns).
                if part in ("all", "hi"):
                    for d in range(NDC):
                        xht, _ = xts[d]
                        st = (d == 0)
                        if qown:
                            nc.tensor.matmul(g1[:], wa1[:, d, :], xht[:, sl],
                                             start=st, stop=False)
                            nc.tensor.matmul(g1[:], wa3[:, d, :], xht[:, sl],
                                             start=False, stop=False)
                            nc.tensor.matmul(vps[:], wv[:, d, :], xht[:, sl],
                                             start=st, stop=(d == NDC - 1))
                        else:
                            nc.tensor.matmul(g1[:], wkv[:, d, :], xht[:, sl],
                                             start=st, stop=False)
                            nc.tensor.matmul(g1[0:64, :], wa3[:, d, 0:64], xht[:, sl],
                                             start=False, stop=False)
                if part == "hi":
                    return
                for d in range(NDC):
                    _, xlt = xts[d]
                    if qown:
                        nc.tensor.matmul(g1[:], wa1[:, d, :], xlt[:, sl],
                                         start=False, stop=(d == NDC - 1))
                    else:
                        nc.tensor.matmul(g1[:], wk0[:, d, :], xlt[:, sl],
                                         start=False, stop=(d == NDC - 1))
                gsl = slice(c0 + cc, c0 + cc + 512)
                # K split: Kh (cast) on ScalarE, Kl (residual) on VectorE
                nc.scalar.copy(KhKl[0:64, gsl], g1[0:64, :])
                nc.vector.tensor_sub(KhKl[64:128, gsl], g1[0:64, :], KhKl[0:64, gsl])
                nc.gpsimd.tensor_copy(K65[0:64, gsl], KhKl[0:64, gsl])
                if qown:
                    nc.scalar.copy(VTsb[:, gsl], vps[:])
                    nc.scalar.copy(QhQh[0:64, gsl], g1[64:128, :])
                    nc.gpsimd.tensor_copy(QhQh[64:128, gsl], QhQh[0:64, gsl])
                    nc.vector.tensor_sub(QlM[0:64, gsl], g1[64:128, :], QhQh[0:64, gsl])
                else:
                    nc.scalar.copy(VTsb[:, gsl], g1[64:128, :])
                # V^T -> V' tiles for this 512-chunk (PE transpose, pairs)
                for i in range((c0 + cc) // 256, (c0 + cc + 512) // 256):
                    tt = pp.tile([P, P], BF, tag="pv", name=f"tt_{i}", bufs=2)
                    nc.tensor.transpose(tt[:, 0:64], VTsb[:, (2 * i) * P:(2 * i + 1) * P], idb[0:64, 0:64])
                    nc.tensor.transpose(tt[:, 64:128], VTsb[:, (2 * i + 1) * P:(2 * i + 2) * P], idb[0:64, 0:64])
                    nc.scalar.copy(V65[:, 2 * i:2 * i + 2, 0:64],
                                   tt[:].rearrange("p (a b) -> p a b", a=2))

            def stage_b_range(rng, plo, phi):
                for p in range(plo, phi):
                    stage_b_p(rng, p)

            def stage_b_p(rng, p, tag="s"):
                base = 0 if rng == 0 else QC
                mw = mApd if rng == 0 else mBpd
                mdst = M16a if rng == 0 else M16b
                ext = (p + 1) * P
                qsl = slice(p * P, (p + 1) * P)
                segs = [(o, min(512, ext - o)) for o in range(0, ext, 512)]
                pm = []
                for si, (o, ln) in enumerate(segs):
                    s1 = pp.tile([P, 512], F32, tag=tag, name=f"s1_{rng}_{p}_{si}",
                                 bufs=4 if tag == "s" else 2)
                    nc.tensor.matmul(s1[:, 0:ln], QhQh[0:64, qsl],
                                     KhKl[0:64, base + o:base + o + ln],
                                     start=True, stop=True)
                    if o + ln == ext:  # diagonal-ish tile lives here
                        nc.vector.tensor_add(s1[:, ln - P:ln], s1[:, ln - P:ln],
                                             mw[:, 512:640])
                    if si == 0:
                        nc.vector.tensor_reduce(mdst[:, p:p + 1], s1[:, 0:ln],
                                                axis=mybir.AxisListType.X,
                                                op=mybir.AluOpType.max)
                    else:
                        rm = ptp.tile([P, 1], F32, tag="rm")
                        nc.vector.tensor_reduce(rm[:], s1[:, 0:ln],
                                                axis=mybir.AxisListType.X,
                                                op=mybir.AluOpType.max)
                        pm.append(rm)
                for rm in pm:
                    nc.vector.tensor_max(mdst[:, p:p + 1], mdst[:, p:p + 1], rm[:])

            def m_finalize(half):
                # -m for q-tiles [8h, 8h+8) -> row 64 of QlM cols [1024h, +1024)
                psl = slice(half * 8, half * 8 + 8)
                nc.vector.tensor_max(M16a[:, psl], M16a[:, psl], M16b[:, psl])
                nc.vector.tensor_scalar_mul(M16[:, psl], M16a[:, psl], -1.0)
                mt = pp.tile([QT, P], F32, tag="s", name=f"mt_{half}", bufs=4)
                nc.tensor.transpose(mt[0:8, :], M16[:, psl], idf[:])
                m16t = m16t0 if half == 0 else m16t1
                nc.vector.tensor_copy(m16t[:], mt[0:8, :])
                nc.gpsimd.dma_start(QlM[64:65, half * 1024:(half + 1) * 1024],
                                    m16t[:])

            def c_tlist(qh):
                qb = qh * 1024
                return [t for t in range(KT) if (t % QT) * P < qb + 1024]

            class CStream:
                """Chunk-level stage-C pipeline. A unit is one (half, key
                tile, 512-col chunk): s3 matmul pair -> optional mask add ->
                exp -> pv accumulate into the per-(half,bank) pv tile. Units
                are ordered so each pv bank closes as early as possible; at a
                bank's last writer the pvsb copy and that bank's four output
                tiles are emitted, overlapping the rest of the stream."""

                def __init__(self, alt=False):
                    self.units = []     # (qh, t, q0, cc, ln)
                    self.pv = {}
                    self.wl = {}        # (qh,bank) -> unit idx list
                    self.pend = []
                    self.p1done = {}
                    self.k = 0
                    self.alt = alt      # also use the idle proj ring for s3
                    self.LA = CFG.get("c_la", 3) * 2

                def add_half(self, qh):
                    qb = qh * 1024
                    tl = [t for t in range(KT) if (t % QT) * P < qb + 1024]
                    qz = {t: max((t % QT) * P, qb) - qb for t in tl}
                    units = [(qh, t, qz[t], qz[t], 512 - qz[t])
                             for t in tl if qz[t] < 512]
                    units += [(qh, t, qz[t], max(qz[t], 512),
                               1024 - max(qz[t], 512)) for t in tl]
                    for u in units:
                        i = len(self.units)
                        self.units.append(u)
                        self.wl.setdefault((qh, u[3] // 512), []).append(i)

                def _p1(self, i):
                    qh, t, q0, cc, ln = self.units[i]
                    qb = qh * 1024
                    ksl = slice(t * P, (t + 1) * P)
                    tg = "proj" if (self.alt and i % CFG.get("altm", 3) == CFG.get("altm", 3) - 1) else "s"
                    s3 = pp.tile([P, 512], F32, tag=tg, name=f"s3_{i}",
                                 bufs=4 if tg == "s" else 2)
                    nc.tensor.matmul(s3[:, 0:ln], KhKl[:, ksl],
                                     QhQh[:, qb + cc:qb + cc + ln],
                                     start=True, stop=False)
                    return s3

                def warm(self, n):
                    # pre-emit pass-1 matmuls (no QlM dependency) so PE has
                    # work while the -m row of the next half is finalized
                    for i in range(self.k, min(self.k + n, len(self.units))):
                        if i not in self.p1done:
                            self.p1done[i] = self._p1(i)

                def _score(self, i):
                    qh, t, q0, cc, ln = self.units[i]
                    qb = qh * 1024
                    ksl = slice(t * P, (t + 1) * P)
                    s3 = self.p1done.pop(i, None)
                    if s3 is None:
                        s3 = self._p1(i)
                    nc.tensor.matmul(s3[:, 0:ln], K65[:, ksl],
                                     QlM[:, qb + cc:qb + cc + ln],
                                     start=False, stop=True)
                    pt = ptp.tile([P, 512], BF, tag="pt", bufs=8)
                    nc.scalar.activation(pt[:, 0:ln], s3[:, 0:ln],
                                         mybir.ActivationFunctionType.Exp,
                                         scale=SCALE)
                    if cc == q0 and (t % QT) * P >= qb:  # q0-block lives here
                        # causal mask applied post-exp on Pool (idle engine):
                        # REPLACE semantics, so masked inf values cannot NaN
                        if t < QT:   # diag tile: zero where k > q
                            nc.gpsimd.affine_select(
                                out=pt[:, 0:P], in_=pt[:, 0:P],
                                compare_op=mybir.AluOpType.is_ge, fill=0.0,
                                base=0, pattern=[[1, P]],
                                channel_multiplier=-1)
                        else:        # boundary tile: all-or-nothing per core
                            nc.gpsimd.tensor_scalar_min(pt[:, 0:P],
                                                        pt[:, 0:P], mbig[:])
                    return pt

                def _pv(self, i, pt):
                    qh, t, q0, cc, ln = self.units[i]
                    bk = (qh, cc // 512)
                    if bk not in self.pv:
                        self.pv[bk] = pp.tile([65, 512], F32, tag="pv",
                                              name=f"pv_{bk[0]}_{bk[1]}", bufs=2)
                    pv = self.pv[bk]
                    o = cc - bk[1] * 512
                    nc.tensor.matmul(pv[:, o:o + ln], V65[:, t, :], pt[:, 0:ln],
                                     start=(i == self.wl[bk][0]),
                                     stop=(i == self.wl[bk][-1]))
                    if i == self.wl[bk][-1]:
                        c0 = qh * 1024 + bk[1] * 512
                        if qh == 0:   # DVE is the hot engine in C0's window
                            nc.scalar.copy(pvsb[:, c0:c0 + 512], pv[:])
                        else:         # Act is the hot engine in C1's window
                            nc.vector.tensor_copy(pvsb[:, c0:c0 + 512], pv[:])
                        final_half(qh, jsub=range(qh * 8 + bk[1] * 4,
                                                  qh * 8 + bk[1] * 4 + 4))

                def emit_one(self):
                    if self.k < len(self.units):
                        i = self.k
                        self.pend.append((i, self._score(i)))
                        self.k += 1
                        if len(self.pend) > self.LA:
                            j, pt = self.pend.pop(0)
                            self._pv(j, pt)
                        return True
                    return False

                def finish(self):
                    while self.emit_one():
                        pass
                    for j, pt in self.pend:
                        self._pv(j, pt)
                    self.pend = []


            def late_consts():
                nc.scalar.dma_start(mApd[:], mApd_d[:])
                nc.scalar.dma_start(mBpd[:], mBpd_d[:])
                nc.scalar.dma_start(mbig[:], mbig_d[:])
                make_identity(nc, idf[:])
                nc.gpsimd.memset(K65[64:65, :], 1.0)
                nc.gpsimd.memset(V65[:, :, 64:65], 1.0)

            def final_half(qh, jsub=None):
                # per 128-row block: transpose out^T, 1/Z, scale, store.
                # qh=0 runs while DVE is hot -> copies/muls on Act; qh=1
                # runs while Act is hot (exp) -> keep them on DVE.
                js = range(qh * 8, qh * 8 + 8) if jsub is None else jsub
                for j in js:
                    tf = pp.tile([P, 65], F32, tag="s", name=f"tf_{j}", bufs=4)
                    nc.tensor.transpose(tf[:, 0:65], pvsb[:, j * P:(j + 1) * P], idf[0:65, 0:65])
                    ot = ptp.tile([P, 65], F32, tag="ot")
                    rz = ptp.tile([P, 1], F32, tag="rz")
                    of = ptp.tile([P, H], F32, tag="of")
                    if qh == 0:
                        nc.scalar.copy(ot[:], tf[:])
                        nc.vector.reciprocal(rz[:], ot[:, 64:65])
                        nc.scalar.mul(of[:], ot[:, 0:64], rz[:])
                    else:
                        nc.vector.tensor_copy(ot[:], tf[:])
                        nc.vector.reciprocal(rz[:], ot[:, 64:65])
                        nc.vector.tensor_scalar_mul(of[:], ot[:, 0:64], rz[:])
                    nc.sync.dma_start(out_d[j * P:(j + 1) * P, :], of[:])

            # ---------------- emission order = scheduler priority ----------
            # head: A quarters against B(r0) + B(r1, 0-8);
            # mid: A3 paired with B(r1, 8-16) (PE vs DVE matched);
            # tail: one long PE-bound C stream, C0 banks then C1 banks,
            # with pv-bank closes cascading into the output tiles.
            stage_a_quarter(0)
            late_consts()
            stage_b_range(0, 0, 8)
            stage_a_quarter(1)
            stage_b_range(0, 8, QT)
            stage_a_quarter(2)
            stage_b_range(1, 0, 8)
            m_finalize(0)
            xts3 = stage_a_dma(3)
            stage_a_chunk(3, 3 * SQ, False, xts3, 0, part="hi")
            stage_a_chunk(3, 3 * SQ, False, xts3, 512, part="hi")
            cs0 = CStream()
            cs0.add_half(0)
            cs1 = CStream(alt=True)
            cs1.add_half(1)
            cs0.emit_one()
            cs0.emit_one()
            stage_a_chunk(3, 3 * SQ, False, xts3, 0, part="lo")
            stage_b_p(1, 8)
            cs0.emit_one()
            stage_b_p(1, 9)
            cs0.emit_one()
            cs0.emit_one()
            stage_a_chunk(3, 3 * SQ, False, xts3, 512, part="lo")
            stage_b_p(1, 10)
            cs0.emit_one()
            cs0.emit_one()
            stage_b_p(1, 11)
            cs0.emit_one()
            for p in range(12, QT):
                stage_b_p(1, p, tag="proj")
                for _ in range(CFG.get("cpb", 2)):
                    cs0.emit_one()
            m_finalize(1)
            cs0.finish()
            cs1.finish()


_NC_CACHE: dict = {}


def _get_nc(reps: int = 1):
    if reps not in _NC_CACHE:
        _NC_CACHE[reps] = _build(reps)
    return _NC_CACHE[reps]


def _host_prep(x, W_query, W_key, W_value):
    """Build the 8 per-core input maps."""
    def split(a):
        hi = a.astype(BF_NP)
        lo = (a - hi.astype(np.float32)).astype(BF_NP)
        return hi, lo

    wq_h, wq_l = split(W_query)
    wk_h, wk_l = split(W_key)
    wv_h, _ = split(W_value)
    def warr(a):  # [D, M] -> SBUF layout [P, NDC*M]
        m = a.shape[1]
        return np.ascontiguousarray(
            a.reshape(NDC, P, m).transpose(1, 0, 2).reshape(P, NDC * m))
    wa1 = warr(np.concatenate([wk_h, wq_h], axis=1))
    wa3 = warr(np.concatenate([wk_l, wq_l], axis=1))
    wkv = warr(np.concatenate([wk_h, wv_h], axis=1))

    r = np.arange(P)
    triA = np.where(r[None, :] <= r[:, None], 0.0, NEG).astype(np.float32)   # [q,k]
    triAC = np.where(r[:, None] <= r[None, :], 0.0, NEG).astype(np.float32)  # [k,q]
    zeros = np.zeros((P, P), np.float32)
    negs = np.full((P, P), NEG, np.float32)
    z512 = np.zeros((P, 512), np.float32)
    mApd = np.concatenate([z512, triA], axis=1)                  # [P, 640]


    perms = []
    for h in range(2):
        perms.append(np.concatenate([np.arange(h, KT, 2), np.arange(1 - h, KT, 2)]))

    in_maps = []
    for c in range(N_CORES):
        b, h = divmod(c, 2)
        xp = x[b].reshape(KT, P, D)[perms[h]].reshape(S, D)
        xt = np.ascontiguousarray(xp.T)
        xt_h = xt.astype(BF_NP)
        xt_l = (xt - xt_h.astype(np.float32)).astype(BF_NP)
        in_maps.append({
            "xh": xt_h, "xl": xt_l,
            "wa1": wa1, "wa3": wa3, "wkv": wkv, "wv": warr(wv_h),
            "mApd": mApd,
            "mBpd": np.concatenate([z512, negs if h == 0 else zeros], axis=1),
            "mbig": np.full((P, 1), 0.0 if h == 0 else 3e38, np.float32),
        })
    return in_maps, perms


def kernel(x, W_query, W_key, W_value, _reps=1):
    x = np.asarray(x, dtype=np.float32)
    W_query = np.asarray(W_query, dtype=np.float32)
    W_key = np.asarray(W_key, dtype=np.float32)
    W_value = np.asarray(W_value, dtype=np.float32)

    in_maps, perms = _host_prep(x, W_query, W_key, W_value)
    nc = _get_nc(_reps)
    res = run_bass_kernel_spmd(nc, in_maps, core_ids=list(range(N_CORES)))

    out = np.empty((B, S, H), np.float32)
    for c in range(N_CORES):
        b, h = divmod(c, 2)
        oc = res.results[c]["out"]          # [2048, 64], permuted query tiles
        for j in range(QT):
            T = perms[h][j]
            out[b, T * P:(T + 1) * P] = oc[j * P:(j + 1) * P]
    return out


# revision 54
# speedup vs baseline: 1.0174x; 1.0174x over previous
"""Causal single-head attention on 8 TRN2 NeuronCores.

Problem: x[4, 4096, 1024], W_q/W_k/W_v [1024, 64] ->
         softmax(causal(q k^T)/8) v   -> [4, 4096, 64]

Sharding: core c = (batch b = c//2, half h = c%2). Each core handles 2048
queries of its batch: h=0 the even 128-row query tiles, h=1 the odd ones
(balanced causal work). The host permutes x's rows per core (own-half tiles
first), so all 8 cores run ONE identical program; the causal structure
differences between halves are encoded in small per-core mask tensors.

Per-core pipeline (all matmuls bf16 hi/lo split, >=16-bit effective):
  A) K^T,Q^T (3-term split, packed [K|Q]) and V^T projections. Quarters 2-3
     don't need Q, so their V rides the [K|V]-packed hi term for free.
  B) max pass: S1 = Qh.Kh per query tile (+ causal masks), row-max m.
  C) value pass: S^T = Qh.Kh + Qh.Kl + Ql.Kh - m computed directly in
     [key, query] layout (the -m rides an appended ones-row of K / -m row
     of Q), exp on ScalarE -> P^T, then out^T = [V|1]^T P^T accumulated on
     PE; the appended ones-column of V yields the softmax denominator Z.
  Final: transpose out^T, multiply by 1/Z, store.

PSUM (8 banks): proj[128,512]x2 (A projections; B's late s1 reuses it) |
s[128,512]x4 (B s1 / C s3 / vps / tf) | pv[65,512]x2 (pv banks + V-
transpose staging). Emission: A quarters run against stage B's DVE-bound
row-max reduces; A3's xl-dependent matmuls are deferred past the DMA
tail; stage C runs as one chunk-level stream (C0 banks then C1 banks)
whose pv banks close early and cascade into the output tiles.
"""
import sys

sys.path.insert(0, "/opt/trn_rl_repo")

import numpy as np
import ml_dtypes

import concourse.bass as bass
import concourse.mybir as mybir
import concourse.tile as tile
from concourse import bacc
from concourse.masks import make_identity
from concourse.bass_utils import run_bass_kernel_spmd

BF = mybir.dt.bfloat16
F32 = mybir.dt.float32
BF_NP = ml_dtypes.bfloat16

P = 128
S = 4096
D = 1024
H = 64
QT = 16            # query tiles per core
KT = 32            # key tiles
QC = QT * P        # queries per core
NDC = D // P       # d-chunks
SCALE = 0.125      # 1/sqrt(64)
NEG = -1e30
N_CORES = 8
B = 4
CFG = {"order": "p2", "c_la": 4, "xin_bufs": 16, "cpb": 4}


def _build(reps: int = 1):
    nc = bacc.Bacc(None, target_bir_lowering=False)
    xh_d = nc.dram_tensor("xh", [D, S], BF, kind="ExternalInput")
    xl_d = nc.dram_tensor("xl", [D, S], BF, kind="ExternalInput")
    wa1_d = nc.dram_tensor("wa1", [P, NDC * P], BF, kind="ExternalInput")  # [Wk_h | Wq_h]
    wa3_d = nc.dram_tensor("wa3", [P, NDC * P], BF, kind="ExternalInput")  # [Wk_l | Wq_l]
    wkv_d = nc.dram_tensor("wkv", [P, NDC * P], BF, kind="ExternalInput")  # [Wk_h | Wv_h]
    wv_d = nc.dram_tensor("wv", [P, NDC * H], BF, kind="ExternalInput")    # Wv_h
    mApd_d = nc.dram_tensor("mApd", [P, 640], F32, kind="ExternalInput")  # [512 zeros | triA]
    mBpd_d = nc.dram_tensor("mBpd", [P, 640], F32, kind="ExternalInput")  # [512 zeros | mB]
    mbig_d = nc.dram_tensor("mbig", [P, 1], F32, kind="ExternalInput")  # 0 (h=0) | 3e38
    out_d = nc.dram_tensor("out", [QC, H], F32, kind="ExternalOutput")

    with tile.TileContext(nc) as tc:
        for _ in range(reps):
            _body(nc, tc, xh_d, xl_d, wa1_d, wa3_d, wkv_d, wv_d,
                  mApd_d, mBpd_d, mbig_d, out_d)
    nc.finalize()
    return nc


def _body(nc, tc, xh_d, xl_d, wa1_d, wa3_d, wkv_d, wv_d,
          mApd_d, mBpd_d, mbig_d, out_d):
    SQ = 1024          # s-quarter width

    with tc.tile_pool(name="pers", bufs=1) as pers:
        # ---------------- persistent SBUF ----------------
        KhKl = pers.tile([P, S], BF)      # rows 0:64 = Kh, 64:128 = Kl
        K65 = pers.tile([65, S], BF)      # Kh + ones row (for -m contraction)
        QhQh = pers.tile([P, QC], BF)     # Qh duplicated on both halves
        QlM = pers.tile([65, QC], BF)     # Ql + (-m) row
        V65 = pers.tile([P, KT, 65], BF)  # V tiles + ones column
        VTsb = pers.tile([H, S], BF)      # V^T staging
        M16a = pers.tile([P, QT], F32)    # col p = range-1 max
        M16b = pers.tile([P, QT], F32)    # col p = range-2 max
        M16 = pers.tile([P, QT], F32)     # col p = -max
        m16t0 = pers.tile([8, P], F32)
        m16t1 = pers.tile([8, P], F32)
        pvsb = pers.tile([65, QC], F32)   # out^T staging
        idf = pers.tile([P, P], F32)
        idb = pers.tile([P, P], BF)
        wa1 = pers.tile([P, NDC, P], BF)
        wa3 = pers.tile([P, NDC, P], BF)
        wkv = pers.tile([P, NDC, P], BF)
        wk0 = pers.tile([P, NDC, P], BF)
        wv = pers.tile([P, NDC, H], BF)
        mApd = pers.tile([P, 640], F32)
        mBpd = pers.tile([P, 640], F32)
        mbig = pers.tile([P, 1], F32)

        nc.scalar.dma_start(wa1[:], wa1_d.rearrange("p (c m) -> p c m", c=NDC))
        nc.scalar.dma_start(wa3[:], wa3_d.rearrange("p (c m) -> p c m", c=NDC))
        nc.scalar.dma_start(wkv[:], wkv_d.rearrange("p (c m) -> p c m", c=NDC))
        nc.scalar.dma_start(wv[:], wv_d.rearrange("p (c m) -> p c m", c=NDC))
        make_identity(nc, idb[:])
        # wk0 = [Wk_h | 0] built on-chip from wa1's left halves
        nc.gpsimd.tensor_copy(wk0[:, :, 0:64], wa1[:, :, 0:64])
        nc.gpsimd.memset(wk0[:, :, 64:128], 0.0)

        # PSUM: proj[128,512]x2 (A) | s[128,512]x4 (B/C/transients) | pv x1.
        with tc.tile_pool(name="pp", bufs=2, space="PSUM") as pp, \
             tc.tile_pool(name="xin", bufs=CFG["xin_bufs"]) as xin, \
             tc.tile_pool(name="ptp", bufs=3) as ptp:

            def stage_a_quarter(sq, cc_filter=None):
                c0 = sq * SQ
                qown = sq < 2      # quarter carries this core's own queries
                xts = stage_a_dma(sq)
                for cc in (0, 512):
                    stage_a_chunk(sq, c0, qown, xts, cc)

            def stage_a_dma(sq):
                # one queue, all xh of the quarter before all xl: the hi-term
                # matmuls (first in each chunk) never wait on xl
                c0 = sq * SQ
                xts = []
                for d in range(NDC):
                    xht = xin.tile([P, SQ], BF, tag="xht", bufs=CFG["xin_bufs"])
                    xlt = xin.tile([P, SQ], BF, tag="xlt", bufs=CFG["xin_bufs"])
                    nc.sync.dma_start(xht[:], xh_d[d * P:(d + 1) * P, c0:c0 + SQ])
                    xts.append((xht, xlt))
                for d in range(NDC):
                    nc.sync.dma_start(xts[d][1][:], xl_d[d * P:(d + 1) * P, c0:c0 + SQ])
                return xts

            _g1c = {}

            def stage_a_chunk(sq, c0, qown, xts, cc, part="all"):
                sl = slice(cc, cc + 512)
                if part in ("all", "hi"):
                    g1 = pp.tile([P, 512], F32, tag="proj", name=f"g1_{sq}_{cc}", bufs=2)
                    vps = (pp.tile([H, 512], F32, tag="s", name=f"vps_{sq}_{cc}",
                                   bufs=4) if qown else None)
                    _g1c[(sq, cc)] = (g1, vps)
                else:
                    g1, vps = _g1c.pop((sq, cc))
                # xh terms for every d-chunk first, then the xl terms.
                # kv-packed quarters: group opens with the 128-wide wkv term
                # and closes with the 128-wide [Wk_h|0] xl term, so start and
                # stop both span the full tile (V half accumulates +0 from
                # the zero columns).
                if part in ("all", "hi"):
                    for d in range(NDC):
                        xht, _ = xts[d]
                        st = (d == 0)
                        if qown:
                            nc.tensor.matmul(g1[:], wa1[:, d, :], xht[:, sl],
                                             start=st, stop=False)
                            nc.tensor.matmul(g1[:], wa3[:, d, :], xht[:, sl],
                                             start=False, stop=False)
                            nc.tensor.matmul(vps[:], wv[:, d, :], xht[:, sl],
                                             start=st, stop=(d == NDC - 1))
                        else:
                            nc.tensor.matmul(g1[:], wkv[:, d, :], xht[:, sl],
                                             start=st, stop=False)
                            nc.tensor.matmul(g1[0:64, :], wa3[:, d, 0:64], xht[:, sl],
                                             start=False, stop=False)
                if part == "hi":
                    return
                for d in range(NDC):
                    _, xlt = xts[d]
                    if qown:
                        nc.tensor.matmul(g1[:], wa1[:, d, :], xlt[:, sl],
                                         start=False, stop=(d == NDC - 1))
                    else:
                        nc.tensor.matmul(g1[:], wk0[:, d, :], xlt[:, sl],
                                         start=False, stop=(d == NDC - 1))
                gsl = slice(c0 + cc, c0 + cc + 512)
                # K split: Kh (cast) on ScalarE, Kl (residual) on VectorE
                nc.scalar.copy(KhKl[0:64, gsl], g1[0:64, :])
                nc.vector.tensor_sub(KhKl[64:128, gsl], g1[0:64, :], KhKl[0:64, gsl])
                nc.gpsimd.tensor_copy(K65[0:64, gsl], KhKl[0:64, gsl])
                if qown:
                    nc.scalar.copy(VTsb[:, gsl], vps[:])
                    nc.scalar.copy(QhQh[0:64, gsl], g1[64:128, :])
                    nc.gpsimd.tensor_copy(QhQh[64:128, gsl], QhQh[0:64, gsl])
                    nc.vector.tensor_sub(QlM[0:64, gsl], g1[64:128, :], QhQh[0:64, gsl])
                else:
                    nc.scalar.copy(VTsb[:, gsl], g1[64:128, :])
                # V^T -> V' tiles for this 512-chunk (PE transpose, pairs)
                for i in range((c0 + cc) // 256, (c0 + cc + 512) // 256):
                    tt = pp.tile([P, P], BF, tag="pv", name=f"tt_{i}", bufs=2)
                    nc.tensor.transpose(tt[:, 0:64], VTsb[:, (2 * i) * P:(2 * i + 1) * P], idb[0:64, 0:64])
                    nc.tensor.transpose(tt[:, 64:128], VTsb[:, (2 * i + 1) * P:(2 * i + 2) * P], idb[0:64, 0:64])
                    nc.scalar.copy(V65[:, 2 * i:2 * i + 2, 0:64],
                                   tt[:].rearrange("p (a b) -> p a b", a=2))

            def stage_b_range(rng, plo, phi):
                for p in range(plo, phi):
                    stage_b_p(rng, p)

            def stage_b_p(rng, p, tag="s"):
                base = 0 if rng == 0 else QC
                mw = mApd if rng == 0 else mBpd
                mdst = M16a if rng == 0 else M16b
                ext = (p + 1) * P
                qsl = slice(p * P, (p + 1) * P)
                segs = [(o, min(512, ext - o)) for o in range(0, ext, 512)]
                pm = []
                for si, (o, ln) in enumerate(segs):
                    s1 = pp.tile([P, 512], F32, tag=tag, name=f"s1_{rng}_{p}_{si}",
                                 bufs=4 if tag == "s" else 2)
                    nc.tensor.matmul(s1[:, 0:ln], QhQh[0:64, qsl],
                                     KhKl[0:64, base + o:base + o + ln],
                                     start=True, stop=True)
                    if o + ln == ext:  # diagonal-ish tile lives here
                        nc.vector.tensor_add(s1[:, ln - P:ln], s1[:, ln - P:ln],
                                             mw[:, 512:640])
                    if si == 0:
                        nc.vector.tensor_reduce(mdst[:, p:p + 1], s1[:, 0:ln],
                                                axis=mybir.AxisListType.X,
                                                op=mybir.AluOpType.max)
                    else:
                        rm = ptp.tile([P, 1], F32, tag="rm")
                        nc.vector.tensor_reduce(rm[:], s1[:, 0:ln],
                                                axis=mybir.AxisListType.X,
                                                op=mybir.AluOpType.max)
                        pm.append(rm)
                for rm in pm:
                    nc.vector.tensor_max(mdst[:, p:p + 1], mdst[:, p:p + 1], rm[:])

            def m_finalize(half):
                # -m for q-tiles [8h, 8h+8) -> row 64 of QlM cols [1024h, +1024)
                psl = slice(half * 8, half * 8 + 8)
                nc.vector.tensor_max(M16a[:, psl], M16a[:, psl], M16b[:, psl])
                nc.vector.tensor_scalar_mul(M16[:, psl], M16a[:, psl], -1.0)
                mt = pp.tile([QT, P], F32, tag="s", name=f"mt_{half}", bufs=4)
                nc.tensor.transpose(mt[0:8, :], M16[:, psl], idf[:])
                m16t = m16t0 if half == 0 else m16t1
                nc.vector.tensor_copy(m16t[:], mt[0:8, :])
                nc.gpsimd.dma_start(QlM[64:65, half * 1024:(half + 1) * 1024],
                                    m16t[:])

            def c_tlist(qh):
                qb = qh * 1024
                return [t for t in range(KT) if (t % QT) * P < qb + 1024]

            class CStream:
                """Chunk-level stage-C pipeline. A unit is one (half, key
                tile, 512-col chunk): s3 matmul pair -> optional mask add ->
                exp -> pv accumulate into the per-(half,bank) pv tile. Units
                are ordered so each pv bank closes as early as possible; at a
                bank's last writer the pvsb copy and that bank's four output
                tiles are emitted, overlapping the rest of the stream."""

                def __init__(self, alt=False):
                    self.units = []     # (qh, t, q0, cc, ln)
                    self.pv = {}
                    self.wl = {}        # (qh,bank) -> unit idx list
                    self.pend = []
                    self.p1done = {}
                    self.k = 0
                    self.alt = alt      # also use the idle proj ring for s3
                    self.LA = CFG.get("c_la", 3) * 2

                def add_half(self, qh):
                    qb = qh * 1024
                    tl = [t for t in range(KT) if (t % QT) * P < qb + 1024]
                    qz = {t: max((t % QT) * P, qb) - qb for t in tl}
                    units = [(qh, t, qz[t], qz[t], 512 - qz[t])
                             for t in tl if qz[t] < 512]
                    units += [(qh, t, qz[t], max(qz[t], 512),
                               1024 - max(qz[t], 512)) for t in tl]
                    for u in units:
                        i = len(self.units)
                        self.units.append(u)
                        self.wl.setdefault((qh, u[3] // 512), []).append(i)

                def _p1(self, i):
                    qh, t, q0, cc, ln = self.units[i]
                    qb = qh * 1024
                    ksl = slice(t * P, (t + 1) * P)
                    tg = "proj" if (self.alt and i % CFG.get("altm", 3) == CFG.get("altm", 3) - 1) else "s"
                    s3 = pp.tile([P, 512], F32, tag=tg, name=f"s3_{i}",
                                 bufs=4 if tg == "s" else 2)
                    nc.tensor.matmul(s3[:, 0:ln], KhKl[:, ksl],
                                     QhQh[:, qb + cc:qb + cc + ln],
                                     start=True, stop=False)
                    return s3

                def warm(self, n):
                    # pre-emit pass-1 matmuls (no QlM dependency) so PE has
                    # work while the -m row of the next half is finalized
                    for i in range(self.k, min(self.k + n, len(self.units))):
                        if i not in self.p1done:
                            self.p1done[i] = self._p1(i)

                def _score(self, i):
                    qh, t, q0, cc, ln = self.units[i]
                    qb = qh * 1024
                    ksl = slice(t * P, (t + 1) * P)
                    s3 = self.p1done.pop(i, None)
                    if s3 is None:
                        s3 = self._p1(i)
                    nc.tensor.matmul(s3[:, 0:ln], K65[:, ksl],
                                     QlM[:, qb + cc:qb + cc + ln],
                                     start=False, stop=True)
                    pt = ptp.tile([P, 512], BF, tag="pt", bufs=8)
                    nc.scalar.activation(pt[:, 0:ln], s3[:, 0:ln],
                                         mybir.ActivationFunctionType.Exp,
                                         scale=SCALE)
                    if cc == q0 and (t % QT) * P >= qb:  # q0-block lives here
                        # causal mask applied post-exp on Pool (idle engine):
                        # REPLACE semantics, so masked inf values cannot NaN
                        if t < QT:   # diag tile: zero where k > q
                            nc.gpsimd.affine_select(
                                out=pt[:, 0:P], in_=pt[:, 0:P],
                                compare_op=mybir.AluOpType.is_ge, fill=0.0,
                                base=0, pattern=[[1, P]],
                                channel_multiplier=-1)
                        else:        # boundary tile: all-or-nothing per core
                            nc.gpsimd.tensor_scalar_min(pt[:, 0:P],
                                                        pt[:, 0:P], mbig[:])
                    return pt

                def _pv(self, i, pt):
                    qh, t, q0, cc, ln = self.units[i]
                    bk = (qh, cc // 512)
                    if bk not in self.pv:
                        self.pv[bk] = pp.tile([65, 512], F32, tag="pv",
                                              name=f"pv_{bk[0]}_{bk[1]}", bufs=2)
                    pv = self.pv[bk]
                    o = cc - bk[1] * 512
                    nc.tensor.matmul(pv[:, o:o + ln], V65[:, t, :], pt[:, 0:ln],
                                     start=(i == self.wl[bk][0]),
                                     stop=(i == self.wl[bk][-1]))
                    if i == self.wl[bk][-1]:
                        c0 = qh * 1024 + bk[1] * 512
                        if qh == 0:   # DVE is the hot engine in C0's window
                            nc.scalar.copy(pvsb[:, c0:c0 + 512], pv[:])
                        else:         # Act is the hot engine in C1's window
                            nc.vector.tensor_copy(pvsb[:, c0:c0 + 512], pv[:])
                        final_half(qh, jsub=range(qh * 8 + bk[1] * 4,
                                                  qh * 8 + bk[1] * 4 + 4))

                def emit_one(self):
                    if self.k < len(self.units):
                        i = self.k
                        self.pend.append((i, self._score(i)))
                        self.k += 1
                        if len(self.pend) > self.LA:
                            j, pt = self.pend.pop(0)
                            self._pv(j, pt)
                        return True
                    return False

                def finish(self):
                    while self.emit_one():
                        pass
                    for j, pt in self.pend:
                        self._pv(j, pt)
                    self.pend = []


            def late_consts():
                nc.scalar.dma_start(mApd[:], mApd_d[:])
                nc.scalar.dma_start(mBpd[:], mBpd_d[:])
                nc.scalar.dma_start(mbig[:], mbig_d[:])
                make_identity(nc, idf[:])
                nc.gpsimd.memset(K65[64:65, :], 1.0)
                nc.gpsimd.memset(V65[:, :, 64:65], 1.0)

            def final_half(qh, jsub=None):
                # per 128-row block: transpose out^T, 1/Z, scale into a
                # 4-block staging tile, then ONE dma for the whole bank.
                # qh=0 runs while DVE is hot -> copies/muls on Act; qh=1
                # runs while Act is hot (exp) -> keep them on DVE.
                js = list(range(qh * 8, qh * 8 + 8) if jsub is None else jsub)
                of4 = ptp.tile([P, len(js), H], F32, tag="of4", bufs=2)
                for idx, j in enumerate(js):
                    tf = pp.tile([P, 65], F32, tag="s", name=f"tf_{j}", bufs=4)
                    nc.tensor.transpose(tf[:, 0:65], pvsb[:, j * P:(j + 1) * P], idf[0:65, 0:65])
                    ot = ptp.tile([P, 65], F32, tag="ot")
                    rz = ptp.tile([P, 1], F32, tag="rz")
                    if qh == 0:
                        nc.scalar.copy(ot[:], tf[:])
                        nc.vector.reciprocal(rz[:], ot[:, 64:65])
                        nc.scalar.mul(of4[:, idx, :], ot[:, 0:64], rz[:])
                    else:
                        nc.vector.tensor_copy(ot[:], tf[:])
                        nc.vector.reciprocal(rz[:], ot[:, 64:65])
                        nc.vector.tensor_scalar_mul(of4[:, idx, :], ot[:, 0:64], rz[:])
                nc.sync.dma_start(
                    out_d[js[0] * P:(js[0] + len(js)) * P, :]
                    .rearrange("(a p) b -> p a b", a=len(js)), of4[:])

            # ---------------- emission order = scheduler priority ----------
            # head: A quarters against B(r0) + B(r1, 0-8);
            # mid: A3 paired with B(r1, 8-16) (PE vs DVE matched);
            # tail: one long PE-bound C stream, C0 banks then C1 banks,
            # with pv-bank closes cascading into the output tiles.
            stage_a_quarter(0)
            late_consts()
            stage_b_range(0, 0, 8)
            stage_a_quarter(1)
            stage_b_range(0, 8, QT)
            stage_a_quarter(2)
            stage_b_range(1, 0, 8)
            m_finalize(0)
            xts3 = stage_a_dma(3)
            stage_a_chunk(3, 3 * SQ, False, xts3, 0, part="hi")
            stage_a_chunk(3, 3 * SQ, False, xts3, 512, part="hi")
            cs0 = CStream()
            cs0.add_half(0)
            cs1 = CStream(alt=True)
            cs1.add_half(1)
            cs0.emit_one()
            cs0.emit_one()
            stage_a_chunk(3, 3 * SQ, False, xts3, 0, part="lo")
            stage_b_p(1, 8)
            cs0.emit_one()
            stage_b_p(1, 9)
            cs0.emit_one()
            cs0.emit_one()
            stage_a_chunk(3, 3 * SQ, False, xts3, 512, part="lo")
            stage_b_p(1, 10)
            cs0.emit_one()
            cs0.emit_one()
            stage_b_p(1, 11)
            cs0.emit_one()
            for p in range(12, QT):
                stage_b_p(1, p, tag="proj")
                for _ in range(CFG.get("cpb", 2)):
                    cs0.emit_one()
            m_finalize(1)
            cs0.finish()
            cs1.finish()


_NC_CACHE: dict = {}


def _get_nc(reps: int = 1):
    if reps not in _NC_CACHE:
        _NC_CACHE[reps] = _build(reps)
    return _NC_CACHE[reps]


def _host_prep(x, W_query, W_key, W_value):
    """Build the 8 per-core input maps."""
    def split(a):
        hi = a.astype(BF_NP)
        lo = (a - hi.astype(np.float32)).astype(BF_NP)
        return hi, lo

    wq_h, wq_l = split(W_query)
    wk_h, wk_l = split(W_key)
    wv_h, _ = split(W_value)
    def warr(a):  # [D, M] -> SBUF layout [P, NDC*M]
        m = a.shape[1]
        return np.ascontiguousarray(
            a.reshape(NDC, P, m).transpose(1, 0, 2).reshape(P, NDC * m))
    wa1 = warr(np.concatenate([wk_h, wq_h], axis=1))
    wa3 = warr(np.concatenate([wk_l, wq_l], axis=1))
    wkv = warr(np.concatenate([wk_h, wv_h], axis=1))

    r = np.arange(P)
    triA = np.where(r[None, :] <= r[:, None], 0.0, NEG).astype(np.float32)   # [q,k]
    triAC = np.where(r[:, None] <= r[None, :], 0.0, NEG).astype(np.float32)  # [k,q]
    zeros = np.zeros((P, P), np.float32)
    negs = np.full((P, P), NEG, np.float32)
    z512 = np.zeros((P, 512), np.float32)
    mApd = np.concatenate([z512, triA], axis=1)                  # [P, 640]


    perms = []
    for h in range(2):
        perms.append(np.concatenate([np.arange(h, KT, 2), np.arange(1 - h, KT, 2)]))

    in_maps = []
    for c in range(N_CORES):
        b, h = divmod(c, 2)
        xp = x[b].reshape(KT, P, D)[perms[h]].reshape(S, D)
        xt = np.ascontiguousarray(xp.T)
        xt_h = xt.astype(BF_NP)
        xt_l = (xt - xt_h.astype(np.float32)).astype(BF_NP)
        in_maps.append({
            "xh": xt_h, "xl": xt_l,
            "wa1": wa1, "wa3": wa3, "wkv": wkv, "wv": warr(wv_h),
            "mApd": mApd,
            "mBpd": np.concatenate([z512, negs if h == 0 else zeros], axis=1),
            "mbig": np.full((P, 1), 0.0 if h == 0 else 3e38, np.float32),
        })
    return in_maps, perms


def kernel(x, W_query, W_key, W_value, _reps=1):
    x = np.asarray(x, dtype=np.float32)
    W_query = np.asarray(W_query, dtype=np.float32)
    W_key = np.asarray(W_key, dtype=np.float32)
    W_value = np.asarray(W_value, dtype=np.float32)

    in_maps, perms = _host_prep(x, W_query, W_key, W_value)
    nc = _get_nc(_reps)
    res = run_bass_kernel_spmd(nc, in_maps, core_ids=list(range(N_CORES)))

    out = np.empty((B, S, H), np.float32)
    for c in range(N_CORES):
        b, h = divmod(c, 2)
        oc = res.results[c]["out"]          # [2048, 64], permuted query tiles
        for j in range(QT):
            T = perms[h][j]
            out[b, T * P:(T + 1) * P] = oc[j * P:(j + 1) * P]
    return out


# revision 56
# speedup vs baseline: 1.0197x; 1.0022x over previous
"""Causal single-head attention on 8 TRN2 NeuronCores.

Problem: x[4, 4096, 1024], W_q/W_k/W_v [1024, 64] ->
         softmax(causal(q k^T)/8) v   -> [4, 4096, 64]

Sharding: core c = (batch b = c//2, half h = c%2). Each core handles 2048
queries of its batch: h=0 the even 128-row query tiles, h=1 the odd ones
(balanced causal work). The host permutes x's rows per core (own-half tiles
first), so all 8 cores run ONE identical program; the causal structure
differences between halves are encoded in small per-core mask tensors.

Per-core pipeline (all matmuls bf16 hi/lo split, >=16-bit effective):
  A) K^T,Q^T (3-term split, packed [K|Q]) and V^T projections. Quarters 2-3
     don't need Q, so their V rides the [K|V]-packed hi term for free.
  B) max pass: S1 = Qh.Kh per query tile (+ causal masks), row-max m.
  C) value pass: S^T = Qh.Kh + Qh.Kl + Ql.Kh - m computed directly in
     [key, query] layout (the -m rides an appended ones-row of K / -m row
     of Q), exp on ScalarE -> P^T, then out^T = [V|1]^T P^T accumulated on
     PE; the appended ones-column of V yields the softmax denominator Z.
  Final: transpose out^T, multiply by 1/Z, store.

PSUM (8 banks): proj[128,512]x2 (A projections; B's late s1 reuses it) |
s[128,512]x4 (B s1 / C s3 / vps / tf) | pv[65,512]x2 (pv banks + V-
transpose staging). Emission: A quarters run against stage B's DVE-bound
row-max reduces; A3's xl-dependent matmuls are deferred past the DMA
tail; stage C runs as one chunk-level stream (C0 banks then C1 banks)
whose pv banks close early and cascade into the output tiles.
"""
import sys

sys.path.insert(0, "/opt/trn_rl_repo")

import numpy as np
import ml_dtypes

import concourse.bass as bass
import concourse.mybir as mybir
import concourse.tile as tile
from concourse import bacc
from concourse.masks import make_identity
from concourse.bass_utils import run_bass_kernel_spmd

BF = mybir.dt.bfloat16
F32 = mybir.dt.float32
BF_NP = ml_dtypes.bfloat16

P = 128
S = 4096
D = 1024
H = 64
QT = 16            # query tiles per core
KT = 32            # key tiles
QC = QT * P        # queries per core
NDC = D // P       # d-chunks
SCALE = 0.125      # 1/sqrt(64)
NEG = -1e30
N_CORES = 8
B = 4
CFG = {"order": "p2", "c_la": 4, "xin_bufs": 16, "cpb": 4, "ptb": 10, "otb": 4}


def _build(reps: int = 1):
    nc = bacc.Bacc(None, target_bir_lowering=False)
    xh_d = nc.dram_tensor("xh", [D, S], BF, kind="ExternalInput")
    xl_d = nc.dram_tensor("xl", [D, S], BF, kind="ExternalInput")
    wa1_d = nc.dram_tensor("wa1", [P, NDC * P], BF, kind="ExternalInput")  # [Wk_h | Wq_h]
    wa3_d = nc.dram_tensor("wa3", [P, NDC * P], BF, kind="ExternalInput")  # [Wk_l | Wq_l]
    wkv_d = nc.dram_tensor("wkv", [P, NDC * P], BF, kind="ExternalInput")  # [Wk_h | Wv_h]
    wv_d = nc.dram_tensor("wv", [P, NDC * H], BF, kind="ExternalInput")    # Wv_h
    mApd_d = nc.dram_tensor("mApd", [P, 640], F32, kind="ExternalInput")  # [512 zeros | triA]
    mBpd_d = nc.dram_tensor("mBpd", [P, 640], F32, kind="ExternalInput")  # [512 zeros | mB]
    mbig_d = nc.dram_tensor("mbig", [P, 1], F32, kind="ExternalInput")  # 0 (h=0) | 3e38
    out_d = nc.dram_tensor("out", [QC, H], F32, kind="ExternalOutput")

    with tile.TileContext(nc) as tc:
        for _ in range(reps):
            _body(nc, tc, xh_d, xl_d, wa1_d, wa3_d, wkv_d, wv_d,
                  mApd_d, mBpd_d, mbig_d, out_d)
    nc.finalize()
    return nc


def _body(nc, tc, xh_d, xl_d, wa1_d, wa3_d, wkv_d, wv_d,
          mApd_d, mBpd_d, mbig_d, out_d):
    SQ = 1024          # s-quarter width

    with tc.tile_pool(name="pers", bufs=1) as pers:
        # ---------------- persistent SBUF ----------------
        KhKl = pers.tile([P, S], BF)      # rows 0:64 = Kh, 64:128 = Kl
        K65 = pers.tile([65, S], BF)      # Kh + ones row (for -m contraction)
        QhQh = pers.tile([P, QC], BF)     # Qh duplicated on both halves
        QlM = pers.tile([65, QC], BF)     # Ql + (-m) row
        V65 = pers.tile([P, KT, 65], BF)  # V tiles + ones column
        VTsb = pers.tile([H, S], BF)      # V^T staging
        M16a = pers.tile([P, QT], F32)    # col p = range-1 max
        M16b = pers.tile([P, QT], F32)    # col p = range-2 max
        M16 = pers.tile([P, QT], F32)     # col p = -max
        m16t0 = pers.tile([8, P], F32)
        m16t1 = pers.tile([8, P], F32)
        pvsb = pers.tile([65, QC], F32)   # out^T staging
        idf = pers.tile([P, P], F32)
        idb = pers.tile([P, P], BF)
        wa1 = pers.tile([P, NDC, P], BF)
        wa3 = pers.tile([P, NDC, P], BF)
        wkv = pers.tile([P, NDC, P], BF)
        wk0 = pers.tile([P, NDC, P], BF)
        wv = pers.tile([P, NDC, H], BF)
        mApd = pers.tile([P, 640], F32)
        mBpd = pers.tile([P, 640], F32)
        mbig = pers.tile([P, 1], F32)

        nc.scalar.dma_start(wa1[:], wa1_d.rearrange("p (c m) -> p c m", c=NDC))
        nc.scalar.dma_start(wa3[:], wa3_d.rearrange("p (c m) -> p c m", c=NDC))
        nc.scalar.dma_start(wkv[:], wkv_d.rearrange("p (c m) -> p c m", c=NDC))
        nc.scalar.dma_start(wv[:], wv_d.rearrange("p (c m) -> p c m", c=NDC))
        make_identity(nc, idb[:])
        # wk0 = [Wk_h | 0] built on-chip from wa1's left halves
        nc.gpsimd.tensor_copy(wk0[:, :, 0:64], wa1[:, :, 0:64])
        nc.gpsimd.memset(wk0[:, :, 64:128], 0.0)

        # PSUM: proj[128,512]x2 (A) | s[128,512]x4 (B/C/transients) | pv x1.
        with tc.tile_pool(name="pp", bufs=2, space="PSUM") as pp, \
             tc.tile_pool(name="xin", bufs=CFG["xin_bufs"]) as xin, \
             tc.tile_pool(name="ptp", bufs=3) as ptp:

            def stage_a_quarter(sq, cc_filter=None):
                c0 = sq * SQ
                qown = sq < 2      # quarter carries this core's own queries
                xts = stage_a_dma(sq)
                for cc in (0, 512):
                    stage_a_chunk(sq, c0, qown, xts, cc)

            def stage_a_dma(sq):
                # one queue, all xh of the quarter before all xl: the hi-term
                # matmuls (first in each chunk) never wait on xl
                c0 = sq * SQ
                xts = []
                for d in range(NDC):
                    xht = xin.tile([P, SQ], BF, tag="xht", bufs=CFG["xin_bufs"])
                    xlt = xin.tile([P, SQ], BF, tag="xlt", bufs=CFG["xin_bufs"])
                    nc.sync.dma_start(xht[:], xh_d[d * P:(d + 1) * P, c0:c0 + SQ])
                    xts.append((xht, xlt))
                for d in range(NDC):
                    nc.sync.dma_start(xts[d][1][:], xl_d[d * P:(d + 1) * P, c0:c0 + SQ])
                return xts

            _g1c = {}

            def stage_a_chunk(sq, c0, qown, xts, cc, part="all"):
                sl = slice(cc, cc + 512)
                if part in ("all", "hi"):
                    g1 = pp.tile([P, 512], F32, tag="proj", name=f"g1_{sq}_{cc}", bufs=2)
                    vps = (pp.tile([H, 512], F32, tag="s", name=f"vps_{sq}_{cc}",
                                   bufs=4) if qown else None)
                    _g1c[(sq, cc)] = (g1, vps)
                else:
                    g1, vps = _g1c.pop((sq, cc))
                # xh terms for every d-chunk first, then the xl terms.
                # kv-packed quarters: group opens with the 128-wide wkv term
                # and closes with the 128-wide [Wk_h|0] xl term, so start and
                # stop both span the full tile (V half accumulates +0 from
                # the zero columns).
                if part in ("all", "hi"):
                    for d in range(NDC):
                        xht, _ = xts[d]
                        st = (d == 0)
                        if qown:
                            nc.tensor.matmul(g1[:], wa1[:, d, :], xht[:, sl],
                                             start=st, stop=False)
                            nc.tensor.matmul(g1[:], wa3[:, d, :], xht[:, sl],
                                             start=False, stop=False)
                            nc.tensor.matmul(vps[:], wv[:, d, :], xht[:, sl],
                                             start=st, stop=(d == NDC - 1))
                        else:
                            nc.tensor.matmul(g1[:], wkv[:, d, :], xht[:, sl],
                                             start=st, stop=False)
                            nc.tensor.matmul(g1[0:64, :], wa3[:, d, 0:64], xht[:, sl],
                                             start=False, stop=False)
                if part == "hi":
                    return
                for d in range(NDC):
                    _, xlt = xts[d]
                    if qown:
                        nc.tensor.matmul(g1[:], wa1[:, d, :], xlt[:, sl],
                                         start=False, stop=(d == NDC - 1))
                    else:
                        nc.tensor.matmul(g1[:], wk0[:, d, :], xlt[:, sl],
                                         start=False, stop=(d == NDC - 1))
                gsl = slice(c0 + cc, c0 + cc + 512)
                # K split: Kh (cast) on ScalarE, Kl (residual) on VectorE
                nc.scalar.copy(KhKl[0:64, gsl], g1[0:64, :])
                nc.vector.tensor_sub(KhKl[64:128, gsl], g1[0:64, :], KhKl[0:64, gsl])
                nc.gpsimd.tensor_copy(K65[0:64, gsl], KhKl[0:64, gsl])
                if qown:
                    nc.scalar.copy(VTsb[:, gsl], vps[:])
                    nc.scalar.copy(QhQh[0:64, gsl], g1[64:128, :])
                    nc.gpsimd.tensor_copy(QhQh[64:128, gsl], QhQh[0:64, gsl])
                    nc.vector.tensor_sub(QlM[0:64, gsl], g1[64:128, :], QhQh[0:64, gsl])
                else:
                    nc.scalar.copy(VTsb[:, gsl], g1[64:128, :])
                # V^T -> V' tiles for this 512-chunk (PE transpose, pairs)
                for i in range((c0 + cc) // 256, (c0 + cc + 512) // 256):
                    tt = pp.tile([P, P], BF, tag="pv", name=f"tt_{i}", bufs=2)
                    nc.tensor.transpose(tt[:, 0:64], VTsb[:, (2 * i) * P:(2 * i + 1) * P], idb[0:64, 0:64])
                    nc.tensor.transpose(tt[:, 64:128], VTsb[:, (2 * i + 1) * P:(2 * i + 2) * P], idb[0:64, 0:64])
                    nc.scalar.copy(V65[:, 2 * i:2 * i + 2, 0:64],
                                   tt[:].rearrange("p (a b) -> p a b", a=2))

            def stage_b_range(rng, plo, phi):
                for p in range(plo, phi):
                    stage_b_p(rng, p)

            def stage_b_p(rng, p, tag="s"):
                base = 0 if rng == 0 else QC
                mw = mApd if rng == 0 else mBpd
                mdst = M16a if rng == 0 else M16b
                ext = (p + 1) * P
                qsl = slice(p * P, (p + 1) * P)
                segs = [(o, min(512, ext - o)) for o in range(0, ext, 512)]
                pm = []
                for si, (o, ln) in enumerate(segs):
                    s1 = pp.tile([P, 512], F32, tag=tag, name=f"s1_{rng}_{p}_{si}",
                                 bufs=4 if tag == "s" else 2)
                    nc.tensor.matmul(s1[:, 0:ln], QhQh[0:64, qsl],
                                     KhKl[0:64, base + o:base + o + ln],
                                     start=True, stop=True)
                    if o + ln == ext:  # diagonal-ish tile lives here
                        nc.vector.tensor_add(s1[:, ln - P:ln], s1[:, ln - P:ln],
                                             mw[:, 512:640])
                    if si == 0:
                        nc.vector.tensor_reduce(mdst[:, p:p + 1], s1[:, 0:ln],
                                                axis=mybir.AxisListType.X,
                                                op=mybir.AluOpType.max)
                    else:
                        rm = ptp.tile([P, 1], F32, tag="rm")
                        nc.vector.tensor_reduce(rm[:], s1[:, 0:ln],
                                                axis=mybir.AxisListType.X,
                                                op=mybir.AluOpType.max)
                        pm.append(rm)
                for rm in pm:
                    nc.vector.tensor_max(mdst[:, p:p + 1], mdst[:, p:p + 1], rm[:])

            def m_finalize(half):
                # -m for q-tiles [8h, 8h+8) -> row 64 of QlM cols [1024h, +1024)
                psl = slice(half * 8, half * 8 + 8)
                nc.vector.tensor_max(M16a[:, psl], M16a[:, psl], M16b[:, psl])
                nc.vector.tensor_scalar_mul(M16[:, psl], M16a[:, psl], -1.0)
                mt = pp.tile([QT, P], F32, tag="s", name=f"mt_{half}", bufs=4)
                nc.tensor.transpose(mt[0:8, :], M16[:, psl], idf[:])
                m16t = m16t0 if half == 0 else m16t1
                nc.vector.tensor_copy(m16t[:], mt[0:8, :])
                nc.gpsimd.dma_start(QlM[64:65, half * 1024:(half + 1) * 1024],
                                    m16t[:])

            def c_tlist(qh):
                qb = qh * 1024
                return [t for t in range(KT) if (t % QT) * P < qb + 1024]

            class CStream:
                """Chunk-level stage-C pipeline. A unit is one (half, key
                tile, 512-col chunk): s3 matmul pair -> optional mask add ->
                exp -> pv accumulate into the per-(half,bank) pv tile. Units
                are ordered so each pv bank closes as early as possible; at a
                bank's last writer the pvsb copy and that bank's four output
                tiles are emitted, overlapping the rest of the stream."""

                def __init__(self, alt=False):
                    self.units = []     # (qh, t, q0, cc, ln)
                    self.pv = {}
                    self.wl = {}        # (qh,bank) -> unit idx list
                    self.pend = []
                    self.p1done = {}
                    self.k = 0
                    self.alt = alt      # also use the idle proj ring for s3
                    self.LA = CFG.get("c_la", 3) * 2

                def add_half(self, qh):
                    qb = qh * 1024
                    tl = [t for t in range(KT) if (t % QT) * P < qb + 1024]
                    qz = {t: max((t % QT) * P, qb) - qb for t in tl}
                    units = [(qh, t, qz[t], qz[t], 512 - qz[t])
                             for t in tl if qz[t] < 512]
                    units += [(qh, t, qz[t], max(qz[t], 512),
                               1024 - max(qz[t], 512)) for t in tl]
                    for u in units:
                        i = len(self.units)
                        self.units.append(u)
                        self.wl.setdefault((qh, u[3] // 512), []).append(i)

                def _p1(self, i):
                    qh, t, q0, cc, ln = self.units[i]
                    qb = qh * 1024
                    ksl = slice(t * P, (t + 1) * P)
                    tg = "proj" if (self.alt and i % CFG.get("altm", 3) == CFG.get("altm", 3) - 1) else "s"
                    s3 = pp.tile([P, 512], F32, tag=tg, name=f"s3_{i}",
                                 bufs=4 if tg == "s" else 2)
                    nc.tensor.matmul(s3[:, 0:ln], KhKl[:, ksl],
                                     QhQh[:, qb + cc:qb + cc + ln],
                                     start=True, stop=False)
                    return s3

                def warm(self, n):
                    # pre-emit pass-1 matmuls (no QlM dependency) so PE has
                    # work while the -m row of the next half is finalized
                    for i in range(self.k, min(self.k + n, len(self.units))):
                        if i not in self.p1done:
                            self.p1done[i] = self._p1(i)

                def _score(self, i):
                    qh, t, q0, cc, ln = self.units[i]
                    qb = qh * 1024
                    ksl = slice(t * P, (t + 1) * P)
                    s3 = self.p1done.pop(i, None)
                    if s3 is None:
                        s3 = self._p1(i)
                    nc.tensor.matmul(s3[:, 0:ln], K65[:, ksl],
                                     QlM[:, qb + cc:qb + cc + ln],
                                     start=False, stop=True)
                    pt = ptp.tile([P, 512], BF, tag="pt", bufs=CFG.get("ptb", 8))
                    nc.scalar.activation(pt[:, 0:ln], s3[:, 0:ln],
                                         mybir.ActivationFunctionType.Exp,
                                         scale=SCALE)
                    if cc == q0 and (t % QT) * P >= qb:  # q0-block lives here
                        # causal mask applied post-exp on Pool (idle engine):
                        # REPLACE semantics, so masked inf values cannot NaN
                        if t < QT:   # diag tile: zero where k > q
                            nc.gpsimd.affine_select(
                                out=pt[:, 0:P], in_=pt[:, 0:P],
                                compare_op=mybir.AluOpType.is_ge, fill=0.0,
                                base=0, pattern=[[1, P]],
                                channel_multiplier=-1)
                        else:        # boundary tile: all-or-nothing per core
                            nc.gpsimd.tensor_scalar_min(pt[:, 0:P],
                                                        pt[:, 0:P], mbig[:])
                    return pt

                def _pv(self, i, pt):
                    qh, t, q0, cc, ln = self.units[i]
                    bk = (qh, cc // 512)
                    if bk not in self.pv:
                        self.pv[bk] = pp.tile([65, 512], F32, tag="pv",
                                              name=f"pv_{bk[0]}_{bk[1]}", bufs=2)
                    pv = self.pv[bk]
                    o = cc - bk[1] * 512
                    nc.tensor.matmul(pv[:, o:o + ln], V65[:, t, :], pt[:, 0:ln],
                                     start=(i == self.wl[bk][0]),
                                     stop=(i == self.wl[bk][-1]))
                    if i == self.wl[bk][-1]:
                        c0 = qh * 1024 + bk[1] * 512
                        if qh == 0:   # DVE is the hot engine in C0's window
                            nc.scalar.copy(pvsb[:, c0:c0 + 512], pv[:])
                        else:         # Act is the hot engine in C1's window
                            nc.vector.tensor_copy(pvsb[:, c0:c0 + 512], pv[:])
                        final_half(qh, jsub=range(qh * 8 + bk[1] * 4,
                                                  qh * 8 + bk[1] * 4 + 4))

                def emit_one(self):
                    if self.k < len(self.units):
                        i = self.k
                        self.pend.append((i, self._score(i)))
                        self.k += 1
                        if len(self.pend) > self.LA:
                            j, pt = self.pend.pop(0)
                            self._pv(j, pt)
                        return True
                    return False

                def finish(self):
                    while self.emit_one():
                        pass
                    for j, pt in self.pend:
                        self._pv(j, pt)
                    self.pend = []


            def late_consts():
                nc.scalar.dma_start(mApd[:], mApd_d[:])
                nc.scalar.dma_start(mBpd[:], mBpd_d[:])
                nc.scalar.dma_start(mbig[:], mbig_d[:])
                make_identity(nc, idf[:])
                nc.gpsimd.memset(K65[64:65, :], 1.0)
                nc.gpsimd.memset(V65[:, :, 64:65], 1.0)

            def final_half(qh, jsub=None):
                # per 128-row block: transpose out^T, 1/Z, scale into a
                # 4-block staging tile, then ONE dma for the whole bank.
                # qh=0 runs while DVE is hot -> copies/muls on Act; qh=1
                # runs while Act is hot (exp) -> keep them on DVE.
                js = list(range(qh * 8, qh * 8 + 8) if jsub is None else jsub)
                of4 = ptp.tile([P, len(js), H], F32, tag="of4", bufs=2)
                for idx, j in enumerate(js):
                    tf = pp.tile([P, 65], F32, tag="s", name=f"tf_{j}", bufs=4)
                    nc.tensor.transpose(tf[:, 0:65], pvsb[:, j * P:(j + 1) * P], idf[0:65, 0:65])
                    ot = ptp.tile([P, 65], F32, tag="ot", bufs=CFG.get("otb", 3))
                    rz = ptp.tile([P, 1], F32, tag="rz", bufs=CFG.get("otb", 3))
                    if qh == 0:
                        nc.scalar.copy(ot[:], tf[:])
                        nc.vector.reciprocal(rz[:], ot[:, 64:65])
                        nc.scalar.mul(of4[:, idx, :], ot[:, 0:64], rz[:])
                    else:
                        nc.vector.tensor_copy(ot[:], tf[:])
                        nc.vector.reciprocal(rz[:], ot[:, 64:65])
                        nc.vector.tensor_scalar_mul(of4[:, idx, :], ot[:, 0:64], rz[:])
                nc.sync.dma_start(
                    out_d[js[0] * P:(js[0] + len(js)) * P, :]
                    .rearrange("(a p) b -> p a b", a=len(js)), of4[:])

            # ---------------- emission order = scheduler priority ----------
            # head: A quarters against B(r0) + B(r1, 0-8);
            # mid: A3 paired with B(r1, 8-16) (PE vs DVE matched);
            # tail: one long PE-bound C stream, C0 banks then C1 banks,
            # with pv-bank closes cascading into the output tiles.
            stage_a_quarter(0)
            late_consts()
            stage_b_range(0, 0, 8)
            stage_a_quarter(1)
            stage_b_range(0, 8, QT)
            stage_a_quarter(2)
            stage_b_range(1, 0, 8)
            m_finalize(0)
            xts3 = stage_a_dma(3)
            stage_a_chunk(3, 3 * SQ, False, xts3, 0, part="hi")
            stage_a_chunk(3, 3 * SQ, False, xts3, 512, part="hi")
            cs0 = CStream()
            cs0.add_half(0)
            cs1 = CStream(alt=True)
            cs1.add_half(1)
            cs0.emit_one()
            cs0.emit_one()
            stage_a_chunk(3, 3 * SQ, False, xts3, 0, part="lo")
            stage_b_p(1, 8)
            cs0.emit_one()
            stage_b_p(1, 9)
            cs0.emit_one()
            cs0.emit_one()
            stage_a_chunk(3, 3 * SQ, False, xts3, 512, part="lo")
            stage_b_p(1, 10)
            cs0.emit_one()
            cs0.emit_one()
            stage_b_p(1, 11)
            cs0.emit_one()
            for p in range(12, QT):
                stage_b_p(1, p, tag="proj")
                for _ in range(CFG.get("cpb", 2)):
                    cs0.emit_one()
            m_finalize(1)
            cs0.finish()
            cs1.finish()


_NC_CACHE: dict = {}


def _get_nc(reps: int = 1):
    if reps not in _NC_CACHE:
        _NC_CACHE[reps] = _build(reps)
    return _NC_CACHE[reps]


def _host_prep(x, W_query, W_key, W_value):
    """Build the 8 per-core input maps."""
    def split(a):
        hi = a.astype(BF_NP)
        lo = (a - hi.astype(np.float32)).astype(BF_NP)
        return hi, lo

    wq_h, wq_l = split(W_query)
    wk_h, wk_l = split(W_key)
    wv_h, _ = split(W_value)
    def warr(a):  # [D, M] -> SBUF layout [P, NDC*M]
        m = a.shape[1]
        return np.ascontiguousarray(
            a.reshape(NDC, P, m).transpose(1, 0, 2).reshape(P, NDC * m))
    wa1 = warr(np.concatenate([wk_h, wq_h], axis=1))
    wa3 = warr(np.concatenate([wk_l, wq_l], axis=1))
    wkv = warr(np.concatenate([wk_h, wv_h], axis=1))

    r = np.arange(P)
    triA = np.where(r[None, :] <= r[:, None], 0.0, NEG).astype(np.float32)   # [q,k]
    triAC = np.where(r[:, None] <= r[None, :], 0.0, NEG).astype(np.float32)  # [k,q]
    zeros = np.zeros((P, P), np.float32)
    negs = np.full((P, P), NEG, np.float32)
    z512 = np.zeros((P, 512), np.float32)
    mApd = np.concatenate([z512, triA], axis=1)                  # [P, 640]


    perms = []
    for h in range(2):
        perms.append(np.concatenate([np.arange(h, KT, 2), np.arange(1 - h, KT, 2)]))

    in_maps = []
    for c in range(N_CORES):
        b, h = divmod(c, 2)
        xp = x[b].reshape(KT, P, D)[perms[h]].reshape(S, D)
        xt = np.ascontiguousarray(xp.T)
        xt_h = xt.astype(BF_NP)
        xt_l = (xt - xt_h.astype(np.float32)).astype(BF_NP)
        in_maps.append({
            "xh": xt_h, "xl": xt_l,
            "wa1": wa1, "wa3": wa3, "wkv": wkv, "wv": warr(wv_h),
            "mApd": mApd,
            "mBpd": np.concatenate([z512, negs if h == 0 else zeros], axis=1),
            "mbig": np.full((P, 1), 0.0 if h == 0 else 3e38, np.float32),
        })
    return in_maps, perms


def kernel(x, W_query, W_key, W_value, _reps=1):
    x = np.asarray(x, dtype=np.float32)
    W_query = np.asarray(W_query, dtype=np.float32)
    W_key = np.asarray(W_key, dtype=np.float32)
    W_value = np.asarray(W_value, dtype=np.float32)

    in_maps, perms = _host_prep(x, W_query, W_key, W_value)
    nc = _get_nc(_reps)
    res = run_bass_kernel_spmd(nc, in_maps, core_ids=list(range(N_CORES)))

    out = np.empty((B, S, H), np.float32)
    for c in range(N_CORES):
        b, h = divmod(c, 2)
        oc = res.results[c]["out"]          # [2048, 64], permuted query tiles
        for j in range(QT):
            T = perms[h][j]
            out[b, T * P:(T + 1) * P] = oc[j * P:(j + 1) * P]
    return out
